# revision 21
# baseline (speedup 1.0000x reference)
"""ArcFace loss (margin softmax CE) on 8 TRN2 NeuronCores.

Strategy (model-parallel softmax CE, classes sharded over 8 cores):
  - host: shard W row-wise by class (12500/core, zero-padded to 12544),
    transpose to [512, Cp] and cast bf16; gather wl = w[labels] (layout
    prep only - all math runs on device).
  - device (SPMD, identical graph on all cores):
      * normalize feats; build fnT (d-major) via PE transposes.
      * per-class weight norms via ones-matmul over squared wT chunks;
        inv = 1/max(sqrt(n2),1e-12) via ACT Sqrt + DVE reciprocal;
        normalize the resident wT in SBUF.
      * main: Z[n,c-chunk] = fnT.T @ wnT (bf16, PSUM f32); fused
        ACT exp(S*z - 64) with accum_out giving per-row partial
        softmax sums (fixed max 64 = S*max|cos| keeps everything
        finite: terms are exp(logit-64) <= 1).
      * label-margin path computed redundantly on every core from wl:
        cosl = <fn, wl/||wl||> rowwise, target logit with ArcFace
        margin, delta = exp(S*t-64) - exp(S*cosl-64).
      * AllReduce(add) the [1024] partial sums; each core finishes:
        loss = mean(64 + ln(P + delta) - S*t).
"""

import math
import os

import numpy as np
import ml_dtypes

import concourse.bass as bass
import concourse.tile as tile
from concourse import bacc, mybir
from concourse.bass import ts, ds
from concourse.bass_utils import run_bass_kernel_spmd
from concourse.masks import make_identity

FP = mybir.dt.float32
BF = mybir.dt.bfloat16
AF = mybir.ActivationFunctionType
OP = mybir.AluOpType

# problem constants (hardcoded per harness contract)
MARGIN = 0.5
S = 64.0
COS_M = math.cos(MARGIN)
SIN_M = math.sin(MARGIN)
MIN_COS = math.cos(math.pi - MARGIN)
C = 100000
D = 512
N = 1024
NCORES = 8
CS = C // NCORES          # 12500 classes per core
CP = 12544                # padded: 98 * 128
NT = N // 128             # 8 row tiles
FB = -64.0                # fixed log-domain shift (= -S * max cos)

# class chunks along the free dim: 24 x 512 + 1 x 256
CHUNKS = [(i * 512, 512) for i in range(24)] + [(24 * 512, 256)]
NCH = len(CHUNKS)

# debug bisection flags
DBG_SKIP_MAIN = bool(int(os.environ.get("DBG_SKIP_MAIN", "0")))
DBG_SKIP_NORM = bool(int(os.environ.get("DBG_SKIP_NORM", "0")))
DBG_SKIP_LBL = bool(int(os.environ.get("DBG_SKIP_LBL", "0")))
DBG_SKIP_CC = bool(int(os.environ.get("DBG_SKIP_CC", "0")))
DBG_NT = int(os.environ.get("DBG_NT", str(NT)))
DBG_DUMP = bool(int(os.environ.get("DBG_DUMP", "0")))


def build_nc():
    nc = bacc.Bacc(
        "TRN2",
        target_bir_lowering=False,
        debug=False,
        enable_asserts=False,
        num_devices=NCORES,
    )

    # activation-bias constants must be pre-registered as const APs
    for val in (1e-24, FB):
        t = nc.alloc_sbuf_tensor(f"const-f32-{val}", [128, 1], FP)
        nc.gpsimd.memset(t.ap(), val)
        nc.const_aps.aps[(FP, val)] = t.ap()
    nc.all_engine_barrier()

    wt_d = nc.dram_tensor("wt", [D, CP], BF, kind="ExternalInput")
    feats_d = nc.dram_tensor("feats", [N, D], FP, kind="ExternalInput")
    wl_d = nc.dram_tensor("wl", [N, D], FP, kind="ExternalInput")
    out_d = nc.dram_tensor("out", [1, 1], FP, kind="ExternalOutput")
    if DBG_DUMP:
        dbg_pall = nc.dram_tensor("dbg_pall", [128, NT], FP, kind="ExternalOutput")
        dbg_pg = nc.dram_tensor("dbg_pg", [128, NT], FP, kind="ExternalOutput")
        dbg_cosl = nc.dram_tensor("dbg_cosl", [128, NT], FP, kind="ExternalOutput")
        dbg_tlog = nc.dram_tensor("dbg_tlog", [128, NT], FP, kind="ExternalOutput")
        dbg_delta = nc.dram_tensor("dbg_delta", [128, NT], FP, kind="ExternalOutput")
        dbg_inv = nc.dram_tensor("dbg_inv", [128, 98], FP, kind="ExternalOutput")
        dbg_rows = nc.dram_tensor("dbg_rows", [128, NCH], FP, kind="ExternalOutput")
        dbg_lnu = nc.dram_tensor("dbg_lnu", [128, NT], FP, kind="ExternalOutput")
        dbg_nll = nc.dram_tensor("dbg_nll", [128, NT], FP, kind="ExternalOutput")
        dbg_nsum = nc.dram_tensor("dbg_nsum", [128, 1], FP, kind="ExternalOutput")

    n2_d = nc.dram_tensor("n2scratch", [1, CP], FP)
    inv_d = nc.dram_tensor("invscratch", [1, CP], BF)
    cc_in = nc.dram_tensor("cc_in", [N], FP)
    cc_out = nc.dram_tensor("cc_out", [N], FP)

    # [128, 98] strided views of the per-class scratch vectors (c = g*128 + p)
    n2_grid = n2_d.ap().rearrange("a (g p) -> (a p) g", p=128)
    inv_grid = inv_d.ap().rearrange("a (g p) -> (a p) g", p=128)
    ccin_grid = cc_in.ap().rearrange("(t p) -> p t", p=128)
    ccout_grid = cc_out.ap().rearrange("(t p) -> p t", p=128)

    with tile.TileContext(nc) as tc, (
        tc.tile_pool(name="const", bufs=1)
    ) as constp, (
        tc.tile_pool(name="wres", bufs=1)
    ) as wres, (
        tc.tile_pool(name="fres", bufs=1)
    ) as fres, (
        tc.tile_pool(name="small", bufs=1)
    ) as small:
        with (
            tc.tile_pool(name="work", bufs=3) as work,
            tc.tile_pool(name="psum0", bufs=2, space="PSUM") as psum0,
            tc.tile_pool(name="psumn2", bufs=2, space="PSUM") as psumn2,
        ):
            identity = constp.tile([128, 128], BF, tag="identity")
            make_identity(nc, identity[:])
            ones_bf = constp.tile([128, 1], BF, tag="ones_bf")
            nc.vector.memset(ones_bf[:], 1.0)
            ones_fp = constp.tile([128, 1], FP, tag="ones_fp")
            nc.vector.memset(ones_fp[:], 1.0)

            # ---- W resident in SBUF: 4 d-chunks x [128, CP] bf16 ----
            wsb = []
            for j in range(4):
                wj = wres.tile([128, CP], BF, tag=f"wsb{j}", name=f"wsb{j}")
                wsb.append(wj)
                # 4 sub-DMAs per chunk for pipelining
                for q in range(4):
                    c0 = q * (CP // 4)
                    nc.sync.dma_start(
                        out=wj[:, ds(c0, CP // 4)],
                        in_=wt_d.ap()[ts(j, 128), ds(c0, CP // 4)],
                    )

            # ---- feats prep: normalize rows, build fnT via PE transpose ----
            fn32 = []   # normalized feats, f32, natural layout (for label path)
            fnT = [
                fres.tile([128, N], BF, tag=f"fnT{j}", name=f"fnT{j}")
                for j in range(4)
            ]
            for t in range(NT):
                f_t = work.tile([128, D], FP, tag="f_t")
                nc.sync.dma_start(out=f_t[:], in_=feats_d.ap()[ts(t, 128), :])
                dump = work.tile([128, D], FP, tag="dump")
                ssq = small.tile([128, NT], FP, tag="ssq")
                nc.vector.scalar_tensor_tensor(
                    out=dump[:], in0=f_t[:], scalar=1.0, in1=f_t[:],
                    op0=OP.mult, op1=OP.mult,
                    accum_out=ssq[:, ts(t, 1)],
                )
                nrm = small.tile([128, NT], FP, tag="fnrm")
                nc.scalar.activation(nrm[:, ts(t, 1)], ssq[:, ts(t, 1)],
                                     AF.Sqrt, bias=1e-24, scale=1.0)
                nc.vector.tensor_scalar_max(nrm[:, ts(t, 1)], nrm[:, ts(t, 1)], 1e-12)
                inv_f = small.tile([128, NT], FP, tag="finv")
                nc.vector.reciprocal(inv_f[:, ts(t, 1)], nrm[:, ts(t, 1)])

                fn_t = fres.tile([128, D], FP, tag=f"fn32_{t}")
                fn32.append(fn_t)
                nc.vector.tensor_scalar_mul(fn_t[:], f_t[:], inv_f[:, ts(t, 1)])
                fnb_t = work.tile([128, D], BF, tag="fnb_t")
                nc.vector.tensor_scalar_mul(fnb_t[:], f_t[:], inv_f[:, ts(t, 1)])
                for j in range(4):
                    tp = psum0.tile([128, 128], BF, tag="tp")
                    nc.tensor.transpose(tp[:], fnb_t[:, ts(j, 128)], identity[:])
                    nc.vector.tensor_copy(fnT[j][:, ts(t, 128)], tp[:])

            # ---- per-class weight norms: n2[c] = sum_d wT[d,c]^2 ----
            for ch, (c0, csz) in enumerate(CHUNKS if not DBG_SKIP_NORM else []):
                n2p = psumn2.tile([1, 512], FP, tag="n2p")
                for j in range(4):
                    wsq = work.tile([128, 512], BF, tag="wsq")
                    nc.vector.tensor_mul(wsq[:, :csz], wsb[j][:, ds(c0, csz)],
                                         wsb[j][:, ds(c0, csz)])
                    nc.tensor.matmul(
                        n2p[:, :csz], ones_bf[:], wsq[:, :csz],
                        start=(j == 0), stop=(j == 3),
                    )
                n2c = work.tile([1, 512], FP, tag="n2c")
                nc.scalar.copy(n2c[:, :csz], n2p[:, :csz])
                nc.sync.dma_start(out=n2_d.ap()[:, ds(c0, csz)], in_=n2c[:, :csz])

            # inv = 1 / max(sqrt(n2), 1e-12), computed in [128, 98] layout
            if not DBG_SKIP_NORM:
                n2g = work.tile([128, 98], FP, tag="n2g")
                nc.gpsimd.dma_start(out=n2g[:], in_=n2_grid)
                sng = work.tile([128, 98], FP, tag="sng")
                nc.scalar.activation(sng[:], n2g[:], AF.Sqrt, bias=1e-24, scale=1.0)
                nc.vector.tensor_scalar_max(sng[:], sng[:], 1e-12)
                invg = work.tile([128, 98], FP, tag="invg")
                nc.vector.reciprocal(invg[:], sng[:])
                if DBG_DUMP:
                    nc.sync.dma_start(out=dbg_inv.ap(), in_=invg[:])
                invgb = work.tile([128, 98], BF, tag="invgb")
                nc.vector.tensor_copy(invgb[:], invg[:])
                nc.gpsimd.dma_start(out=inv_grid, in_=invgb[:])

                # normalize resident W chunk-by-chunk (broadcast inv along d)
                for ch, (c0, csz) in enumerate(CHUNKS):
                    invb = work.tile([128, 512], BF, tag="invb")
                    nc.gpsimd.dma_start(
                        out=invb[:, :csz],
                        in_=inv_d.ap()[:, ds(c0, csz)].broadcast_to([128, csz]),
                    )
                    for j in range(4):
                        nc.vector.tensor_mul(wsb[j][:, ds(c0, csz)],
                                             wsb[j][:, ds(c0, csz)], invb[:, :csz])

            # ---- label-margin path (redundant on every core) ----
            cosl = small.tile([128, NT], FP, tag="cosl")
            if DBG_SKIP_LBL:
                nc.vector.memset(cosl[:], 0.0)
            for t in range(NT if not DBG_SKIP_LBL else 0):
                wl_t = work.tile([128, D], FP, tag="wl_t")
                nc.sync.dma_start(out=wl_t[:], in_=wl_d.ap()[ts(t, 128), :])
                dump2 = work.tile([128, D], FP, tag="dump2")
                wsql = small.tile([128, NT], FP, tag="wsql")
                nc.vector.scalar_tensor_tensor(
                    out=dump2[:], in0=wl_t[:], scalar=1.0, in1=wl_t[:],
                    op0=OP.mult, op1=OP.mult,
                    accum_out=wsql[:, ts(t, 1)],
                )
                wnrm = small.tile([128, NT], FP, tag="wnrm")
                nc.scalar.activation(wnrm[:, ts(t, 1)], wsql[:, ts(t, 1)],
                                     AF.Sqrt, bias=1e-24, scale=1.0)
                nc.vector.tensor_scalar_max(wnrm[:, ts(t, 1)], wnrm[:, ts(t, 1)], 1e-12)
                winv = small.tile([128, NT], FP, tag="winv")
                nc.vector.reciprocal(winv[:, ts(t, 1)], wnrm[:, ts(t, 1)])
                wln_t = work.tile([128, D], FP, tag="wln_t")
                nc.vector.tensor_scalar_mul(wln_t[:], wl_t[:], winv[:, ts(t, 1)])
                dump3 = work.tile([128, D], FP, tag="dump3")
                nc.vector.scalar_tensor_tensor(
                    out=dump3[:], in0=fn32[t][:], scalar=1.0, in1=wln_t[:],
                    op0=OP.mult, op1=OP.mult,
                    accum_out=cosl[:, ts(t, 1)],
                )

            # margin math on [128, 8]
            nc.vector.tensor_scalar(cosl[:], cosl[:], -1.0, 1.0, OP.max, OP.min)
            sq = small.tile([128, NT], FP, tag="sq")
            nc.vector.tensor_mul(sq[:], cosl[:], cosl[:])
            sin2 = small.tile([128, NT], FP, tag="sin2")
            nc.vector.tensor_scalar(sin2[:], sq[:], -1.0, 1.0 + 1e-5, OP.mult, OP.add)
            sinl = small.tile([128, NT], FP, tag="sinl")
            nc.scalar.activation(sinl[:], sin2[:], AF.Sqrt, bias=0.0, scale=1.0)
            cosm = small.tile([128, NT], FP, tag="cosm")
            sinm = small.tile([128, NT], FP, tag="sinm")
            nc.vector.tensor_scalar_mul(sinm[:], sinl[:], SIN_M)
            nc.vector.scalar_tensor_tensor(
                out=cosm[:], in0=cosl[:], scalar=COS_M, in1=sinm[:],
                op0=OP.mult, op1=OP.subtract,
            )
            other = small.tile([128, NT], FP, tag="other")
            nc.vector.scalar_tensor_tensor(
                out=other[:], in0=sinl[:], scalar=-MARGIN, in1=cosl[:],
                op0=OP.mult, op1=OP.add,
            )
            mask = small.tile([128, NT], mybir.dt.uint8, tag="mask")
            nc.vector.tensor_single_scalar(mask[:], cosl[:], MIN_COS, OP.is_gt)
            target = small.tile([128, NT], FP, tag="target")
            nc.vector.select(target[:], mask[:], cosm[:], other[:])
            tlog = small.tile([128, NT], FP, tag="tlog")
            nc.vector.tensor_scalar_mul(tlog[:], target[:], S)
            e1 = small.tile([128, NT], FP, tag="e1")
            nc.scalar.activation(e1[:], target[:], AF.Exp, bias=FB, scale=S)
            e2 = small.tile([128, NT], FP, tag="e2")
            nc.scalar.activation(e2[:], cosl[:], AF.Exp, bias=FB, scale=S)
            delta = small.tile([128, NT], FP, tag="delta")
            nc.vector.tensor_sub(delta[:], e1[:], e2[:])

        # ---- main loop: Z = fnT.T @ wnT, fused exp + row-sum ----
        with (
            tc.tile_pool(name="mwork", bufs=4) as mwork,
            tc.tile_pool(name="msmall", bufs=1) as msmall,
            tc.tile_pool(name="zpsum", bufs=4, space="PSUM") as zpsum,
        ):
            pall = msmall.tile([128, NT], FP, tag="pall")
            if DBG_SKIP_MAIN or DBG_NT < NT:
                nc.vector.memset(pall[:], 1.0)
            for t in range(DBG_NT if not DBG_SKIP_MAIN else 0):
                rowsums = msmall.tile([128, NCH], FP, tag=f"rows{t}")
                for ch, (c0, csz) in enumerate(CHUNKS):
                    z = zpsum.tile([128, 512], FP, tag="z")
                    for j in range(4):
                        nc.tensor.matmul(
                            z[:, :csz],
                            fnT[j][:, ts(t, 128)],
                            wsb[j][:, ds(c0, csz)],
                            start=(j == 0), stop=(j == 3),
                        )
                    pd = mwork.tile([128, 512], BF, tag="pd")
                    nc.scalar.activation(
                        pd[:, :csz], z[:, :csz], AF.Exp,
                        bias=FB, scale=S,
                        accum_out=rowsums[:, ts(ch, 1)],
                    )
                if DBG_DUMP and t == 0:
                    nc.sync.dma_start(out=dbg_rows.ap(), in_=rowsums[:])
                nc.vector.tensor_reduce(
                    pall[:, ts(t, 1)], rowsums[:],
                    axis=mybir.AxisListType.X, op=OP.add,
                )

            # ---- all-reduce partial sums, finish loss ----
            pg = msmall.tile([128, NT], FP, tag="pg")
            if DBG_SKIP_CC:
                nc.vector.tensor_scalar_mul(pg[:], pall[:], 8.0)
            else:
                nc.gpsimd.dma_start(out=ccin_grid, in_=pall[:])
                nc.gpsimd.collective_compute(
                    "AllReduce",
                    OP.add,
                    replica_groups=[list(range(NCORES))],
                    ins=[cc_in.ap().opt()],
                    outs=[cc_out.ap().opt()],
                )
                nc.gpsimd.dma_start(out=pg[:], in_=ccout_grid)

            u = msmall.tile([128, NT], FP, tag="u")
            nc.vector.tensor_add(u[:], pg[:], delta[:])
            # HW Ln is inaccurate for tiny args; rescale by an exact 2^70
            # (Ln(U * 2^70) - 70*ln2 == Ln(U)) to keep the arg in [1, 1e6)
            lnu = msmall.tile([128, NT], FP, tag="lnu")
            nc.scalar.activation(lnu[:], u[:], AF.Ln, bias=0.0, scale=2.0 ** 70)
            nll = msmall.tile([128, NT], FP, tag="nll")
            nc.vector.scalar_tensor_tensor(
                out=nll[:], in0=lnu[:], scalar=(-FB - 70.0 * math.log(2.0)),
                in1=tlog[:],
                op0=OP.add, op1=OP.subtract,
            )
            if DBG_DUMP:
                nc.sync.dma_start(out=dbg_pall.ap(), in_=pall[:])
                nc.sync.dma_start(out=dbg_pg.ap(), in_=pg[:])
                nc.sync.dma_start(out=dbg_cosl.ap(), in_=cosl[:])
                nc.sync.dma_start(out=dbg_tlog.ap(), in_=tlog[:])
                nc.sync.dma_start(out=dbg_delta.ap(), in_=delta[:])
            nsum = msmall.tile([128, 1], FP, tag="nsum")
            nc.vector.tensor_reduce(nsum[:], nll[:],
                                    axis=mybir.AxisListType.X, op=OP.add)
            if DBG_DUMP:
                nc.sync.dma_start(out=dbg_lnu.ap(), in_=lnu[:])
                nc.sync.dma_start(out=dbg_nll.ap(), in_=nll[:])
                nc.sync.dma_start(out=dbg_nsum.ap(), in_=nsum[:])
            with tc.tile_pool(name="lpsum", bufs=1, space="PSUM") as lpsum:
                ones_fp2 = msmall.tile([128, 1], FP, tag="ones_fp2")
                nc.vector.memset(ones_fp2[:], 1.0)
                lp = lpsum.tile([1, 1], FP, tag="lp")
                nc.tensor.matmul(lp[:], ones_fp2[:], nsum[:], start=True, stop=True)
                res = msmall.tile([1, 1], FP, tag="res")
                nc.scalar.activation(res[:], lp[:], AF.Copy, bias=0.0, scale=1.0 / N)
                nc.sync.dma_start(out=out_d.ap(), in_=res[:])

    nc.compile()
    return nc


_NC_CACHE = None


def _get_nc():
    global _NC_CACHE
    if _NC_CACHE is None:
        _NC_CACHE = build_nc()
    return _NC_CACHE


def _make_in_maps(feats, w, labels):
    feats = np.asarray(feats, dtype=np.float32).reshape(N, D)
    w = np.asarray(w, dtype=np.float32)
    labels = np.asarray(labels).astype(np.int64)
    wl = np.ascontiguousarray(w[labels]).astype(np.float32)
    in_maps = []
    for i in range(NCORES):
        wt = np.zeros((D, CP), dtype=ml_dtypes.bfloat16)
        wt[:, :CS] = np.ascontiguousarray(
            w[i * CS:(i + 1) * CS].T
        ).astype(ml_dtypes.bfloat16)
        in_maps.append({"wt": wt, "feats": feats, "wl": wl})
    return in_maps


def run(feats, w, labels, trace=False):
    nc = _get_nc()
    in_maps = _make_in_maps(feats, w, labels)
    res = run_bass_kernel_spmd(nc, in_maps, core_ids=list(range(NCORES)),
                               trace=trace)
    out = np.asarray(res.results[0]["out"], dtype=np.float32).reshape(())
    return out, res


def kernel(feats, w, labels):
    out, _ = run(feats, w, labels)
    return out


# revision 23
# speedup vs baseline: 1.0095x; 1.0095x over previous
"""ArcFace loss (margin softmax CE) on 8 TRN2 NeuronCores.

Strategy (model-parallel softmax CE, classes sharded over 8 cores):
  - host: shard W row-wise by class (12500/core, zero-padded to 12544),
    transpose to [512, Cp] and cast bf16; gather wl = w[labels] (layout
    prep only - all math runs on device).
  - device (SPMD, identical graph on all cores):
      * normalize feats; build fnT (d-major) via PE transposes.
      * per-class weight norms via ones-matmul over squared wT chunks,
        pipelined in groups of 5 chunks; inv = exp(-0.5*ln(n2+eps))
        (exp/ln only -> single ACT table set, no Sqrt); normalize the
        SBUF-resident wT group by group so main matmuls start early.
      * main: Z[n,c-chunk] = fnT.T @ wnT (bf16, PSUM f32); fused
        ACT exp(S*z - 64) with accum_out giving per-row partial
        softmax sums (fixed max 64 = S*max|cos| keeps all terms <= 1).
      * label-margin path computed redundantly on every core from wl.
      * AllReduce(add) the [1024] partial sums; each core finishes:
        loss = mean(64 + ln(P + delta) - S*t), with Ln rescaled by an
        exact 2^70 (HW Ln is inaccurate for ~1e-21 args).
"""

import math
import os

import numpy as np
import ml_dtypes

import concourse.bass as bass
import concourse.tile as tile
from concourse import bacc, mybir
from concourse.bass import ts, ds
from concourse.bass_utils import run_bass_kernel_spmd
from concourse.masks import make_identity

FP = mybir.dt.float32
BF = mybir.dt.bfloat16
AF = mybir.ActivationFunctionType
OP = mybir.AluOpType

# problem constants (hardcoded per harness contract)
MARGIN = 0.5
S = 64.0
COS_M = math.cos(MARGIN)
SIN_M = math.sin(MARGIN)
MIN_COS = math.cos(math.pi - MARGIN)
C = 100000
D = 512
N = 1024
NCORES = 8
CS = C // NCORES          # 12500 classes per core
CP = 12544                # padded: 98 * 128
NT = N // 128             # 8 row tiles
FB = -64.0                # fixed log-domain shift (= -S * max cos)
LN2 = math.log(2.0)

# class chunks along the free dim: 24 x 512 + 1 x 256, in groups of 5
CHUNKS = [(i * 512, 512) for i in range(24)] + [(24 * 512, 256)]
NCH = len(CHUNKS)
NGRP = 5
GROUPS = [list(range(g * 5, g * 5 + 5)) for g in range(NGRP)]


def build_nc():
    nc = bacc.Bacc(
        "TRN2",
        target_bir_lowering=False,
        debug=False,
        enable_asserts=False,
        num_devices=NCORES,
    )

    # activation-bias constants must be pre-registered as const APs
    for val in (1e-24, FB):
        t = nc.alloc_sbuf_tensor(f"const-f32-{val}", [128, 1], FP)
        nc.gpsimd.memset(t.ap(), val)
        nc.const_aps.aps[(FP, val)] = t.ap()
    nc.all_engine_barrier()

    wt_d = nc.dram_tensor("wt", [D, CP], BF, kind="ExternalInput")
    feats_d = nc.dram_tensor("feats", [N, D], FP, kind="ExternalInput")
    wl_d = nc.dram_tensor("wl", [N, D], FP, kind="ExternalInput")
    out_d = nc.dram_tensor("out", [1, 1], FP, kind="ExternalOutput")

    n2_d = nc.dram_tensor("n2scratch", [1, CP], FP)
    inv_d = nc.dram_tensor("invscratch", [1, CP], BF)
    cc_in = nc.dram_tensor("cc_in", [N], FP)
    cc_out = nc.dram_tensor("cc_out", [N], FP)

    # [128, 98] strided views of the per-class scratch vectors (c = g*128 + p)
    n2_grid = n2_d.ap().rearrange("a (g p) -> (a p) g", p=128)
    inv_grid = inv_d.ap().rearrange("a (g p) -> (a p) g", p=128)
    ccin_grid = cc_in.ap().rearrange("(t p) -> p t", p=128)
    ccout_grid = cc_out.ap().rearrange("(t p) -> p t", p=128)

    with tile.TileContext(nc) as tc, (
        tc.tile_pool(name="const", bufs=1)
    ) as constp, (
        tc.tile_pool(name="wres", bufs=1)
    ) as wres, (
        tc.tile_pool(name="fres", bufs=1)
    ) as fres, (
        tc.tile_pool(name="small", bufs=1)
    ) as small, (
        tc.tile_pool(name="work", bufs=3)
    ) as work, (
        tc.tile_pool(name="msmall", bufs=1)
    ) as msmall, (
        tc.tile_pool(name="zpsum", bufs=3, space="PSUM")
    ) as zpsum, (
        tc.tile_pool(name="psum0", bufs=2, space="PSUM")
    ) as psum0, (
        tc.tile_pool(name="psumn2", bufs=2, space="PSUM")
    ) as psumn2:
        identity = constp.tile([128, 128], BF, tag="identity")
        make_identity(nc, identity[:])
        ones_bf = constp.tile([128, 1], BF, tag="ones_bf")
        nc.vector.memset(ones_bf[:], 1.0)

        # ---- W load: 4 d-chunks x [128, CP] bf16, DMA'd per group ----
        wsb = []
        for j in range(4):
            wj = wres.tile([128, CP], BF, tag=f"wsb{j}", name=f"wsb{j}")
            wsb.append(wj)
        for g in range(NGRP):
            g0 = CHUNKS[GROUPS[g][0]][0]
            gsz = sum(CHUNKS[ch][1] for ch in GROUPS[g])
            for j in range(4):
                nc.sync.dma_start(
                    out=wsb[j][:, ds(g0, gsz)],
                    in_=wt_d.ap()[ts(j, 128), ds(g0, gsz)],
                )

        # ---- feats prep: normalize rows, build fnT via PE transpose ----
        fn32 = []   # normalized feats, f32, natural layout (label path)
        fnT = [
            fres.tile([128, N], BF, tag=f"fnT{j}", name=f"fnT{j}")
            for j in range(4)
        ]
        for t in range(NT):
            f_t = work.tile([128, D], FP, tag="f_t")
            nc.sync.dma_start(out=f_t[:], in_=feats_d.ap()[ts(t, 128), :])
            dump = work.tile([128, D], FP, tag="dump")
            ssq = small.tile([128, NT], FP, tag="ssq")
            nc.vector.scalar_tensor_tensor(
                out=dump[:], in0=f_t[:], scalar=1.0, in1=f_t[:],
                op0=OP.mult, op1=OP.mult,
                accum_out=ssq[:, ts(t, 1)],
            )
            # 1/||f|| = exp(-0.5*ln(ssq+eps)) -- exp/ln only (one ACT set)
            lnf = small.tile([128, NT], FP, tag="lnf")
            nc.scalar.activation(lnf[:, ts(t, 1)], ssq[:, ts(t, 1)],
                                 AF.Ln, bias=1e-24, scale=1.0)
            inv_f = small.tile([128, NT], FP, tag="finv")
            nc.scalar.activation(inv_f[:, ts(t, 1)], lnf[:, ts(t, 1)],
                                 AF.Exp, bias=0.0, scale=-0.5)

            fn_t = fres.tile([128, D], FP, tag=f"fn32_{t}", name=f"fn32_{t}")
            fn32.append(fn_t)
            nc.vector.tensor_scalar_mul(fn_t[:], f_t[:], inv_f[:, ts(t, 1)])
            fnb_t = work.tile([128, D], BF, tag="fnb_t")
            nc.vector.tensor_scalar_mul(fnb_t[:], f_t[:], inv_f[:, ts(t, 1)])
            for j in range(4):
                tp = psum0.tile([128, 128], BF, tag="tp")
                nc.tensor.transpose(tp[:], fnb_t[:, ts(j, 128)], identity[:])
                nc.vector.tensor_copy(fnT[j][:, ts(t, 128)], tp[:])

        # ---- pipelined: weight norms + normalize per group, then the
        #      main matmuls for that group's chunks (chunk-outer, t-inner)
        pall = msmall.tile([128, NT], FP, tag="pall")
        rows = [
            msmall.tile([128, NCH], FP, tag=f"rows{t}", name=f"rows{t}")
            for t in range(NT)
        ]
        for g in range(NGRP):
            # n2[c] = sum_d wT[d,c]^2 via ones-matmul over squared chunks
            for ch in GROUPS[g]:
                c0, csz = CHUNKS[ch]
                n2p = psumn2.tile([1, 512], FP, tag="n2p")
                for j in range(4):
                    wsq = work.tile([128, 512], BF, tag="wsq")
                    nc.vector.tensor_mul(wsq[:, :csz], wsb[j][:, ds(c0, csz)],
                                         wsb[j][:, ds(c0, csz)])
                    nc.tensor.matmul(
                        n2p[:, :csz], ones_bf[:], wsq[:, :csz],
                        start=(j == 0), stop=(j == 3),
                    )
                n2c = work.tile([1, 512], FP, tag="n2c")
                nc.scalar.copy(n2c[:, :csz], n2p[:, :csz])
                nc.sync.dma_start(out=n2_d.ap()[:, ds(c0, csz)],
                                  in_=n2c[:, :csz])

            # inv = exp(-0.5*ln(n2+eps)) in [128, G] grid layout
            gc0 = CHUNKS[GROUPS[g][0]][0]
            glast0, glastsz = CHUNKS[GROUPS[g][-1]]
            gcols = (glast0 + glastsz - gc0) // 128
            gw0 = gc0 // 128
            n2g = work.tile([128, 20], FP, tag="n2g")
            nc.gpsimd.dma_start(out=n2g[:, :gcols],
                                in_=n2_grid[:, ds(gw0, gcols)])
            lng = work.tile([128, 20], FP, tag="lng")
            nc.scalar.activation(lng[:, :gcols], n2g[:, :gcols],
                                 AF.Ln, bias=1e-24, scale=1.0)
            invgb = work.tile([128, 20], BF, tag="invgb")
            nc.scalar.activation(invgb[:, :gcols], lng[:, :gcols],
                                 AF.Exp, bias=0.0, scale=-0.5)
            nc.gpsimd.dma_start(out=inv_grid[:, ds(gw0, gcols)],
                                in_=invgb[:, :gcols])

            for ch in GROUPS[g]:
                c0, csz = CHUNKS[ch]
                # normalize resident W (broadcast inv along d)
                invb = work.tile([128, 512], BF, tag="invb")
                nc.gpsimd.dma_start(
                    out=invb[:, :csz],
                    in_=inv_d.ap()[:, ds(c0, csz)].broadcast_to([128, csz]),
                )
                for j in range(4):
                    nc.vector.tensor_mul(wsb[j][:, ds(c0, csz)],
                                         wsb[j][:, ds(c0, csz)],
                                         invb[:, :csz])
                # main: Z = fnT.T @ wnT for all row tiles of this chunk
                for t in range(NT):
                    z = zpsum.tile([128, 512], FP, tag="z")
                    for j in range(4):
                        nc.tensor.matmul(
                            z[:, :csz],
                            fnT[j][:, ts(t, 128)],
                            wsb[j][:, ds(c0, csz)],
                            start=(j == 0), stop=(j == 3),
                        )
                    pd = work.tile([128, 512], BF, tag="pd")
                    nc.scalar.activation(
                        pd[:, :csz], z[:, :csz], AF.Exp,
                        bias=FB, scale=S,
                        accum_out=rows[t][:, ts(ch, 1)],
                    )

        for t in range(NT):
            nc.vector.tensor_reduce(
                pall[:, ts(t, 1)], rows[t][:],
                axis=mybir.AxisListType.X, op=OP.add,
            )

        # ---- label-margin path (redundant on every core) ----
        cosl = small.tile([128, NT], FP, tag="cosl")
        for t in range(NT):
            wl_t = work.tile([128, D], FP, tag="wl_t")
            nc.sync.dma_start(out=wl_t[:], in_=wl_d.ap()[ts(t, 128), :])
            dump2 = work.tile([128, D], FP, tag="dump2")
            wsql = small.tile([128, NT], FP, tag="wsql")
            nc.vector.scalar_tensor_tensor(
                out=dump2[:], in0=wl_t[:], scalar=1.0, in1=wl_t[:],
                op0=OP.mult, op1=OP.mult,
                accum_out=wsql[:, ts(t, 1)],
            )
            lnw = small.tile([128, NT], FP, tag="lnw")
            nc.scalar.activation(lnw[:, ts(t, 1)], wsql[:, ts(t, 1)],
                                 AF.Ln, bias=1e-24, scale=1.0)
            winv = small.tile([128, NT], FP, tag="winv")
            nc.scalar.activation(winv[:, ts(t, 1)], lnw[:, ts(t, 1)],
                                 AF.Exp, bias=0.0, scale=-0.5)
            wln_t = work.tile([128, D], FP, tag="wln_t")
            nc.vector.tensor_scalar_mul(wln_t[:], wl_t[:], winv[:, ts(t, 1)])
            dump3 = work.tile([128, D], FP, tag="dump3")
            nc.vector.scalar_tensor_tensor(
                out=dump3[:], in0=fn32[t][:], scalar=1.0, in1=wln_t[:],
                op0=OP.mult, op1=OP.mult,
                accum_out=cosl[:, ts(t, 1)],
            )

        # margin math on [128, 8]
        nc.vector.tensor_scalar(cosl[:], cosl[:], -1.0, 1.0, OP.max, OP.min)
        sq = small.tile([128, NT], FP, tag="sq")
        nc.vector.tensor_mul(sq[:], cosl[:], cosl[:])
        sin2 = small.tile([128, NT], FP, tag="sin2")
        nc.vector.tensor_scalar(sin2[:], sq[:], -1.0, 1.0 + 1e-5,
                                OP.mult, OP.add)
        # sin = exp(0.5*ln(sin2)); sin2 in [~0.9, 1+1e-5] -> Ln accurate
        lns = small.tile([128, NT], FP, tag="lns")
        nc.scalar.activation(lns[:], sin2[:], AF.Ln, bias=0.0, scale=1.0)
        sinl = small.tile([128, NT], FP, tag="sinl")
        nc.scalar.activation(sinl[:], lns[:], AF.Exp, bias=0.0, scale=0.5)
        cosm = small.tile([128, NT], FP, tag="cosm")
        sinm = small.tile([128, NT], FP, tag="sinm")
        nc.vector.tensor_scalar_mul(sinm[:], sinl[:], SIN_M)
        nc.vector.scalar_tensor_tensor(
            out=cosm[:], in0=cosl[:], scalar=COS_M, in1=sinm[:],
            op0=OP.mult, op1=OP.subtract,
        )
        other = small.tile([128, NT], FP, tag="other")
        nc.vector.scalar_tensor_tensor(
            out=other[:], in0=sinl[:], scalar=-MARGIN, in1=cosl[:],
            op0=OP.mult, op1=OP.add,
        )
        mask = small.tile([128, NT], mybir.dt.uint8, tag="mask")
        nc.vector.tensor_single_scalar(mask[:], cosl[:], MIN_COS, OP.is_gt)
        target = small.tile([128, NT], FP, tag="target")
        nc.vector.select(target[:], mask[:], cosm[:], other[:])
        tlog = small.tile([128, NT], FP, tag="tlog")
        nc.vector.tensor_scalar_mul(tlog[:], target[:], S)
        e1 = small.tile([128, NT], FP, tag="e1")
        nc.scalar.activation(e1[:], target[:], AF.Exp, bias=FB, scale=S)
        e2 = small.tile([128, NT], FP, tag="e2")
        nc.scalar.activation(e2[:], cosl[:], AF.Exp, bias=FB, scale=S)
        delta = small.tile([128, NT], FP, tag="delta")
        nc.vector.tensor_sub(delta[:], e1[:], e2[:])

        # ---- all-reduce partial sums, finish loss ----
        nc.gpsimd.dma_start(out=ccin_grid, in_=pall[:])
        nc.gpsimd.collective_compute(
            "AllReduce",
            OP.add,
            replica_groups=[list(range(NCORES))],
            ins=[cc_in.ap().opt()],
            outs=[cc_out.ap().opt()],
        )
        pg = msmall.tile([128, NT], FP, tag="pg")
        nc.gpsimd.dma_start(out=pg[:], in_=ccout_grid)

        u = msmall.tile([128, NT], FP, tag="u")
        nc.vector.tensor_add(u[:], pg[:], delta[:])
        # HW Ln is inaccurate for tiny args; rescale by an exact 2^70
        lnu = msmall.tile([128, NT], FP, tag="lnu")
        nc.scalar.activation(lnu[:], u[:], AF.Ln, bias=0.0, scale=2.0 ** 70)
        nll = msmall.tile([128, NT], FP, tag="nll")
        nc.vector.scalar_tensor_tensor(
            out=nll[:], in0=lnu[:], scalar=(-FB - 70.0 * LN2), in1=tlog[:],
            op0=OP.add, op1=OP.subtract,
        )
        nsum = msmall.tile([128, 1], FP, tag="nsum")
        nc.vector.tensor_reduce(nsum[:], nll[:],
                                axis=mybir.AxisListType.X, op=OP.add)
        ones_fp2 = msmall.tile([128, 1], FP, tag="ones_fp2")
        nc.vector.memset(ones_fp2[:], 1.0)
        lp = psumn2.tile([1, 1], FP, tag="lp", bufs=1)
        nc.tensor.matmul(lp[:], ones_fp2[:], nsum[:], start=True, stop=True)
        res = msmall.tile([1, 1], FP, tag="res")
        nc.scalar.activation(res[:], lp[:], AF.Copy, bias=0.0, scale=1.0 / N)
        nc.sync.dma_start(out=out_d.ap(), in_=res[:])

    nc.compile()
    return nc


_NC_CACHE = None


def _get_nc():
    global _NC_CACHE
    if _NC_CACHE is None:
        _NC_CACHE = build_nc()
    return _NC_CACHE


def _make_in_maps(feats, w, labels):
    feats = np.asarray(feats, dtype=np.float32).reshape(N, D)
    w = np.asarray(w, dtype=np.float32)
    labels = np.asarray(labels).astype(np.int64)
    wl = np.ascontiguousarray(w[labels]).astype(np.float32)
    in_maps = []
    for i in range(NCORES):
        wt = np.zeros((D, CP), dtype=ml_dtypes.bfloat16)
        wt[:, :CS] = np.ascontiguousarray(
            w[i * CS:(i + 1) * CS].T
        ).astype(ml_dtypes.bfloat16)
        in_maps.append({"wt": wt, "feats": feats, "wl": wl})
    return in_maps


def run(feats, w, labels, trace=False):
    nc = _get_nc()
    in_maps = _make_in_maps(feats, w, labels)
    res = run_bass_kernel_spmd(nc, in_maps, core_ids=list(range(NCORES)),
                               trace=trace)
    out = np.asarray(res.results[0]["out"], dtype=np.float32).reshape(())
    return out, res


def kernel(feats, w, labels):
    out, _ = run(feats, w, labels)
    return out
